# revision 1
# baseline (speedup 1.0000x reference)
"""Multi-head cross-attention on 8 Trainium2 NeuronCores.

Problem shapes (hardcoded): B=4, Ld=1024, Le=2048, d_model=1024, 8 heads x 128.
Sharding: core c handles batch b=c//2 and head-group g=c%2 (4 heads each).
Each core computes q/k/v projections for its heads, attention, and a partial
output projection over its heads' value dims; the host sums the two partial
outputs per batch and adds b_o.

All matmuls run as float32r (fp32 storage, full-rate PE streaming).
Softmax denominators come from a ones-column matmul accumulated in PSUM
alongside the attention*V matmul; normalization multiplies by the
partition-broadcast reciprocal.

Phase order is chosen so DMA stays ahead of the PE: the K projection (which
owns the cold start) streams its weight+encoder chunks per contraction step,
later phases' inputs trickle in behind the compute of earlier ones, and the
Q projection runs last on fully resident inputs. PSUM is ping-ponged in
4-bank groups so accumulator copy-backs overlap the next group's matmuls.
The output projection is interleaved per query-half behind the attention
loop.
"""

import math
import sys

import numpy as np

for _p in ("/opt/trn_rl_repo", "/root/.axon_site/_ro/trn_rl_repo"):
    if _p not in sys.path:
        sys.path.append(_p)

B = 4
LQ = 1024
LK = 2048
D = 1024
H = 8
DH = 128
P = 128
HPC = 4          # heads per core
OQ = HPC * DH    # 512 projected dims per core
NQ = 512         # matmul moving free dim
KC = D // P      # 8 contraction chunks for projections
LKC = LK // P    # 16 key chunks
N_CORES = 8

_BUILT = {}


def _build(masked):
    import concourse.bass as bass  # noqa: F401
    import concourse.tile as tile
    import concourse.mybir as mybir
    from concourse import bacc

    f32 = mybir.dt.float32
    f32r = mybir.dt.float32r
    Exp = mybir.ActivationFunctionType.Exp

    nc = bacc.Bacc("TRN2", target_bir_lowering=False, debug=False,
                   num_devices=N_CORES)

    xT = nc.dram_tensor("xT", [D, LQ], f32r, kind="ExternalInput").ap()
    encT = nc.dram_tensor("encT", [D, LK], f32r, kind="ExternalInput").ap()
    wqT = nc.dram_tensor("wqT", [D, OQ], f32r, kind="ExternalInput").ap()
    wkT = nc.dram_tensor("wkT", [D, OQ], f32r, kind="ExternalInput").ap()
    wvT = nc.dram_tensor("wvT", [D, OQ], f32r, kind="ExternalInput").ap()
    woT = nc.dram_tensor("woT", [OQ, D], f32r, kind="ExternalInput").ap()
    bq_d = nc.dram_tensor("bq", [P, HPC], f32, kind="ExternalInput").ap()
    bk_d = nc.dram_tensor("bk", [P, HPC], f32, kind="ExternalInput").ap()
    bv_d = nc.dram_tensor("bv", [P, HPC], f32, kind="ExternalInput").ap()
    ones_d = nc.dram_tensor("ones", [P, 1], f32r, kind="ExternalInput").ap()
    if masked:
        maskT = nc.dram_tensor("maskT", [LK, LQ], f32, kind="ExternalInput").ap()
    out_d = nc.dram_tensor("out", [LQ, D], f32, kind="ExternalOutput").ap()

    HLK = LK // 2  # 1024, one lk-half of the encoder

    with tile.TileContext(nc) as tc:
        with tc.tile_pool(name="persist", bufs=1) as persist:
            qT = [persist.tile([P, LQ], f32r, name=f"qT{h}") for h in range(HPC)]
            kT = [persist.tile([P, LK], f32r, name=f"kT{h}") for h in range(HPC)]
            vch = [persist.tile([P, OQ], f32r, name=f"v{j}") for j in range(LKC)]
            bq_sb = persist.tile([P, HPC], f32, name="bq")
            bk_sb = persist.tile([P, HPC], f32, name="bk")
            bv_sb = persist.tile([P, HPC], f32, name="bv")
            ones_col = persist.tile([P, 1], f32r, name="ones")
            wkc = [persist.tile([P, OQ], f32r, name=f"wk{d}") for d in range(KC)]
            wvc = [persist.tile([P, OQ], f32r, name=f"wv{d}") for d in range(KC)]
            wqc = [persist.tile([P, OQ], f32r, name=f"wq{d}") for d in range(KC)]
            woch = [persist.tile([P, D], f32r, name=f"wo{h}")
                    for h in range(HPC)]

            with (
                tc.tile_pool(name="acc", bufs=1, space="PSUM") as acc,
                tc.tile_pool(name="xh", bufs=6) as xhp,
            ):
                banks = [acc.tile([P, NQ], f32, name=f"bank{t}")
                         for t in range(8)]

                def kproj(e, lh, grp):
                    """kT for one lk-half: grp 0 -> banks 0-3, grp 1 -> 4-7."""
                    l2 = grp
                    for d in range(KC):
                        for h in range(HPC):
                            nc.tensor.matmul(
                                banks[grp * 4 + h][:],
                                wkc[d][:, h * DH:(h + 1) * DH],
                                e[d][:, l2 * NQ:(l2 + 1) * NQ],
                                start=(d == 0), stop=(d == KC - 1))
                    for h in range(HPC):
                        off = lh * HLK + l2 * NQ
                        nc.vector.tensor_scalar_add(
                            kT[h][:, off:off + NQ], banks[grp * 4 + h][:],
                            bk_sb[:, h:h + 1])

                def vproj(e, lh, grp):
                    """v chunks j = lh*8 + grp*4 ... +4."""
                    for d in range(KC):
                        for jj in range(4):
                            jloc = grp * 4 + jj
                            nc.tensor.matmul(
                                banks[grp * 4 + jj][:],
                                e[d][:, jloc * P:(jloc + 1) * P],
                                wvc[d][:],
                                start=(d == 0), stop=(d == KC - 1))
                    for jj in range(4):
                        nc.vector.tensor_copy(
                            vch[lh * 8 + grp * 4 + jj][:],
                            banks[grp * 4 + jj][:])

                def qproj(grp):
                    """qT for query half q2=grp from the streamed x half."""
                    q2 = grp
                    xh = []
                    for d in range(KC):
                        xt = xhp.tile([P, NQ], f32r, name="xh")
                        nc.sync.dma_start(
                            xt[:], xT[d * P:(d + 1) * P,
                                      q2 * NQ:(q2 + 1) * NQ])
                        xh.append(xt)
                        for h in range(HPC):
                            nc.tensor.matmul(
                                banks[grp * 4 + h][:],
                                wqc[d][:, h * DH:(h + 1) * DH],
                                xt[:],
                                start=(d == 0), stop=(d == KC - 1))
                    for h in range(HPC):
                        nc.scalar.add(
                            qT[h][:, q2 * NQ:(q2 + 1) * NQ],
                            banks[grp * 4 + h][:], bq_sb[:, h:h + 1])

                with tc.tile_pool(name="enc0", bufs=1) as enc0p:
                    e0 = [enc0p.tile([P, HLK], f32r, name=f"e0_{d}")
                          for d in range(KC)]
                    # --- K proj, lk-half 0 (cold start: stream wk + e0).
                    for d in range(KC):
                        nc.sync.dma_start(wkc[d][:],
                                          wkT[d * P:(d + 1) * P, :])
                        nc.sync.dma_start(e0[d][:],
                                          encT[d * P:(d + 1) * P, :HLK])
                        if d == 0:
                            nc.sync.dma_start(bq_sb[:], bq_d[:])
                            nc.sync.dma_start(bk_sb[:], bk_d[:])
                            nc.sync.dma_start(bv_sb[:], bv_d[:])
                            nc.sync.dma_start(ones_col[:], ones_d[:])
                    kproj(e0, 0, 0)
                    kproj(e0, 0, 1)
                    # --- V proj, lk-half 0; wv streams in behind.
                    for d in range(KC):
                        nc.sync.dma_start(wvc[d][:],
                                          wvT[d * P:(d + 1) * P, :])
                    vproj(e0, 0, 0)
                    vproj(e0, 0, 1)

                with tc.tile_pool(name="enc1", bufs=1) as enc1p:
                    e1 = [enc1p.tile([P, HLK], f32r, name=f"e1_{d}")
                          for d in range(KC)]
                    # --- K proj, lk-half 1 (e1 streams per d).
                    for d in range(KC):
                        nc.sync.dma_start(e1[d][:],
                                          encT[d * P:(d + 1) * P, HLK:])
                    kproj(e1, 1, 0)
                    kproj(e1, 1, 1)
                    # --- V proj, lk-half 1; wq + wo stream in behind.
                    for d in range(KC):
                        nc.sync.dma_start(wqc[d][:],
                                          wqT[d * P:(d + 1) * P, :])
                        if d % 2 == 0:
                            nc.sync.dma_start(woch[d // 2][:],
                                              woT[(d // 2) * P:
                                                  (d // 2 + 1) * P, :])
                    vproj(e1, 1, 0)
                    vproj(e1, 1, 1)
                    # --- Q proj (x halves stream inside).
                    qproj(0)
                    qproj(1)

            # ---- Attention (q2-outer) + interleaved output projection.
            with tc.tile_pool(name="att", bufs=1) as attp:
                valsT = [attp.tile([P, LQ], f32r, name=f"valsT{h}")
                         for h in range(HPC)]

                with (
                    tc.tile_pool(name="pTp", bufs=8) as pTp,
                    tc.tile_pool(name="smallp", bufs=2) as smallp,
                    tc.tile_pool(name="maskp", bufs=16 if masked else 1) as maskp,
                    tc.tile_pool(name="osb", bufs=4) as osb,
                    tc.tile_pool(name="pss", bufs=3, space="PSUM") as pss,
                    tc.tile_pool(name="psa", bufs=2, space="PSUM") as psa,
                    tc.tile_pool(name="psd", bufs=1, space="PSUM") as psd,
                    tc.tile_pool(name="pso", bufs=2, space="PSUM") as pso,
                ):
                    for q2 in range(LQ // NQ):
                        if masked:
                            mch = []
                            for j in range(LKC):
                                mt = maskp.tile([P, NQ], f32, name=f"m{j}")
                                nc.sync.dma_start(
                                    mt[:],
                                    maskT[j * P:(j + 1) * P,
                                          q2 * NQ:(q2 + 1) * NQ])
                                mch.append(mt)
                        for h in range(HPC):
                            ps_v = psa.tile([P, NQ], f32, name="ps_v")
                            ps_d = psd.tile([1, NQ], f32, name="ps_d")
                            for j in range(LKC):
                                ps_s = pss.tile([P, NQ], f32, name="ps_s")
                                nc.tensor.matmul(
                                    ps_s[:],
                                    kT[h][:, j * P:(j + 1) * P],
                                    qT[h][:, q2 * NQ:(q2 + 1) * NQ],
                                    start=True, stop=True)
                                pT = pTp.tile([P, NQ], f32r, name="pT")
                                if masked:
                                    nc.vector.tensor_add(
                                        ps_s[:], ps_s[:], mch[j][:])
                                nc.scalar.activation(pT[:], ps_s[:], Exp)
                                nc.tensor.matmul(
                                    ps_v[:],
                                    vch[j][:, h * DH:(h + 1) * DH],
                                    pT[:],
                                    start=(j == 0), stop=(j == LKC - 1))
                                nc.tensor.matmul(
                                    ps_d[:],
                                    ones_col[:],
                                    pT[:],
                                    start=(j == 0), stop=(j == LKC - 1))
                            recip = smallp.tile([1, NQ], f32, name="recip")
                            nc.vector.reciprocal(recip[:], ps_d[:])
                            bcast = smallp.tile([P, NQ], f32, name="bcast")
                            nc.gpsimd.partition_broadcast(bcast[:], recip[:])
                            vs = valsT[h][:, q2 * NQ:(q2 + 1) * NQ]
                            nc.vector.tensor_mul(vs, ps_v[:], bcast[:])
                            nc.scalar.add(vs, vs, bv_sb[:, h:h + 1])
                        # Output projection for this query half.
                        for lqc in range(q2 * 4, (q2 + 1) * 4):
                            for o2 in range(D // NQ):
                                po = pso.tile([P, NQ], f32, name="pso")
                                for h in range(HPC):
                                    nc.tensor.matmul(
                                        po[:],
                                        valsT[h][:, lqc * P:(lqc + 1) * P],
                                        woch[h][:, o2 * NQ:(o2 + 1) * NQ],
                                        start=(h == 0), stop=(h == HPC - 1))
                                ot = osb.tile([P, NQ], f32, name="ot")
                                nc.vector.tensor_copy(ot[:], po[:])
                                nc.sync.dma_start(
                                    out_d[lqc * P:(lqc + 1) * P,
                                          o2 * NQ:(o2 + 1) * NQ], ot[:])

    nc.compile()
    return nc


def _get_built(masked):
    if masked not in _BUILT:
        _BUILT[masked] = _build(masked)
    return _BUILT[masked]


def _shard_inputs(inputs, masked):
    x = np.asarray(inputs["mhca_input"], np.float32)
    enc = np.asarray(inputs["encoder_output"], np.float32)
    mask = np.asarray(inputs["cross_mask"], np.float32)
    W_kv = np.asarray(inputs["W_kv"], np.float32)
    b_kv = np.asarray(inputs["b_kv"], np.float32)
    W_q = np.asarray(inputs["W_q"], np.float32)
    b_q = np.asarray(inputs["b_q"], np.float32)
    W_o = np.asarray(inputs["W_o"], np.float32)

    scale = 1.0 / math.sqrt(DH)
    in_maps = []
    for c in range(N_CORES):
        b = c // 2
        g = c % 2
        heads = list(range(g * HPC, (g + 1) * HPC))
        sl = slice(g * OQ, (g + 1) * OQ)
        k_rows = np.concatenate(
            [W_kv[h * 2 * DH:h * 2 * DH + DH] for h in heads], 0)
        v_rows = np.concatenate(
            [W_kv[h * 2 * DH + DH:(h + 1) * 2 * DH] for h in heads], 0)
        m = {
            "xT": np.ascontiguousarray(x[b].T),
            "encT": np.ascontiguousarray(enc[b].T),
            "wqT": np.ascontiguousarray((W_q[sl] * scale).T),
            "wkT": np.ascontiguousarray(k_rows.T),
            "wvT": np.ascontiguousarray(v_rows.T),
            "woT": np.ascontiguousarray(W_o[:, sl].T),
            "bq": np.ascontiguousarray((b_q[sl] * scale).reshape(HPC, DH).T),
            "bk": np.ascontiguousarray(
                np.stack([b_kv[h * 2 * DH:h * 2 * DH + DH] for h in heads], 1)),
            "bv": np.ascontiguousarray(
                np.stack([b_kv[h * 2 * DH + DH:(h + 1) * 2 * DH]
                          for h in heads], 1)),
            "ones": np.ones((P, 1), np.float32),
        }
        if masked:
            m["maskT"] = np.ascontiguousarray(mask[b].T)
        in_maps.append(m)
    return in_maps


def kernel(mhca_input, encoder_output, cross_mask, W_kv, b_kv, W_q, b_q, W_o,
           b_o):
    from concourse.bass_utils import run_bass_kernel_spmd

    inputs = {
        "mhca_input": mhca_input, "encoder_output": encoder_output,
        "cross_mask": cross_mask, "W_kv": W_kv, "b_kv": b_kv, "W_q": W_q,
        "b_q": b_q, "W_o": W_o,
    }
    b_o = np.asarray(b_o, np.float32)
    masked = bool(np.any(np.asarray(cross_mask)))
    nc = _get_built(masked)
    in_maps = _shard_inputs(inputs, masked)

    res = run_bass_kernel_spmd(nc, in_maps, core_ids=list(range(N_CORES)))
    outs = [res.results[c]["out"] for c in range(N_CORES)]
    full = np.stack([outs[2 * b] + outs[2 * b + 1] for b in range(B)], 0)
    return (full + b_o[None, None, :]).astype(np.float32)



# revision 3
# speedup vs baseline: 1.2906x; 1.2906x over previous
"""Multi-head cross-attention on 8 Trainium2 NeuronCores.

Problem shapes (hardcoded): B=4, Ld=1024, Le=2048, d_model=1024, 8 heads x 128.
Sharding: core c handles batch b=c//2 and head-group g=c%2 (4 heads each).
Each core computes q/k/v projections for its heads, attention, and a partial
output projection over its heads' value dims; the host sums the two partial
outputs per batch and adds the bias.

Everything runs in bf16 (inputs converted host-side), matmuls at full PE rate.
Exact algebraic reductions vs the reference:
  - the k bias is dropped: adding q.bk to every score of a query cancels in
    softmax,
  - the v bias folds into a constant output bias (attention weights sum to 1,
    so attn@(v+bv) = attn@v + bv), applied host-side together with b_o,
  - the softmax denominator is computed by the same matmuls as attn@V: the
    moving operand is [v_chunk | ones-column] (129 wide) with exp'd scores as
    the stationary operand, so column 128 of the accumulator is sum(exp) and
    no separate denominator pass is needed.
The fused attn@V produces vals in [q, vd] layout; a DMA-XBAR transpose turns
it into [vd, q] for the output projection, keeping the PE free.

Work is emitted software-pipelined in units of (head, query-half).  Engine
queues are strict FIFO, so emission order is chosen so that no instruction
ever waits on one emitted later: V-projection chunks are front-loaded as
filler behind units 0-1's scores, unit u's fused matmuls are emitted early in
unit u+1 (before u+1's scores can throttle on their exp WAR edges), and the
output projection fills units 5-7.
"""

import math
import sys
from collections import deque

import numpy as np

for _p in ("/opt/trn_rl_repo", "/root/.axon_site/_ro/trn_rl_repo"):
    if _p not in sys.path:
        sys.path.append(_p)

B = 4
LQ = 1024
LK = 2048
D = 1024
H = 8
DH = 128
P = 128
HPC = 4          # heads per core
OQ = HPC * DH    # 512 projected dims per core
KC = D // P      # 8 contraction chunks for projections
LKC = LK // P    # 16 key chunks
QH = 512         # query half
N_CORES = 8

_BUILT = {}


def _build(masked):
    import concourse.bass as bass  # noqa: F401
    import concourse.tile as tile
    import concourse.mybir as mybir
    from concourse import bacc

    f32 = mybir.dt.float32
    bf16 = mybir.dt.bfloat16
    Exp = mybir.ActivationFunctionType.Exp

    nc = bacc.Bacc("TRN2", target_bir_lowering=False, debug=False,
                   num_devices=N_CORES)

    encT = nc.dram_tensor("encT", [D, LK], bf16, kind="ExternalInput").ap()
    xT = nc.dram_tensor("xT", [D, LQ], bf16, kind="ExternalInput").ap()
    wkT = nc.dram_tensor("wkT", [D, OQ], bf16, kind="ExternalInput").ap()
    wvT = nc.dram_tensor("wvT", [D, OQ], bf16, kind="ExternalInput").ap()
    wqT = nc.dram_tensor("wqT", [D, OQ], bf16, kind="ExternalInput").ap()
    woT = nc.dram_tensor("woT", [OQ, D], bf16, kind="ExternalInput").ap()
    bq_d = nc.dram_tensor("bq", [P, HPC], f32, kind="ExternalInput").ap()
    if masked:
        maskT = nc.dram_tensor("maskT", [LK, LQ], bf16,
                               kind="ExternalInput").ap()
    out_d = nc.dram_tensor("out", [LQ, D], bf16, kind="ExternalOutput").ap()

    with tile.TileContext(nc) as tc:
        with tc.tile_pool(name="persist", bufs=1) as persist:
            e = [[persist.tile([P, 1024], bf16, name=f"e{lh}_{d}")
                  for d in range(KC)] for lh in range(2)]
            xc = [persist.tile([P, LQ], bf16, name=f"x{d}") for d in range(KC)]
            wkc = [persist.tile([P, OQ], bf16, name=f"wk{d}") for d in range(KC)]
            wvc = [persist.tile([P, OQ], bf16, name=f"wv{d}") for d in range(KC)]
            wqc = [persist.tile([P, OQ], bf16, name=f"wq{d}") for d in range(KC)]
            woch = [persist.tile([P, D], bf16, name=f"wo{h}")
                    for h in range(HPC)]
            kT = [persist.tile([P, LK], bf16, name=f"kT{h}") for h in range(HPC)]
            qT = [persist.tile([P, LQ], bf16, name=f"qT{h}") for h in range(HPC)]
            # per key-chunk: 4 head blocks of [128 v-dims | ones | pad]
            vch = [persist.tile([P, HPC, 130], bf16, name=f"v{j}")
                   for j in range(LKC)]
            valsT = [persist.tile([P, LQ], bf16, name=f"valsT{h}")
                     for h in range(HPC)]
            bq_sb = persist.tile([P, HPC], f32, name="bq")

            # ---- input DMAs, in consumption order.
            nc.sync.dma_start(bq_sb[:], bq_d[:])
            for d in range(KC):
                nc.sync.dma_start(wkc[d][:], wkT[d * P:(d + 1) * P, :])
                nc.sync.dma_start(e[0][d][:], encT[d * P:(d + 1) * P, :1024])
            for d in range(KC):
                nc.sync.dma_start(e[1][d][:], encT[d * P:(d + 1) * P, 1024:])
            for d in range(KC):
                nc.sync.dma_start(wqc[d][:], wqT[d * P:(d + 1) * P, :])
                nc.sync.dma_start(xc[d][:], xT[d * P:(d + 1) * P, :])
            for d in range(KC):
                nc.sync.dma_start(wvc[d][:], wvT[d * P:(d + 1) * P, :])
            for h in range(HPC):
                nc.sync.dma_start(woch[h][:], woT[h * P:(h + 1) * P, :])

            # ones columns for the fused attn@V / denominator matmuls
            for j in range(LKC):
                nc.vector.memset(vch[j][:], 1.0)

            # ---- K projection then Q projection (4 PSUM banks).
            with tc.tile_pool(name="ppA", bufs=1, space="PSUM") as ppA:
                ab = [ppA.tile([P, 512], f32, name=f"a{t}") for t in range(4)]
                for quarter in range(4):
                    lh, lkq = quarter // 2, quarter % 2
                    for d in range(KC):
                        for h in range(HPC):
                            nc.tensor.matmul(
                                ab[h][:],
                                wkc[d][:, h * DH:(h + 1) * DH],
                                e[lh][d][:, lkq * 512:(lkq + 1) * 512],
                                start=(d == 0), stop=(d == KC - 1))
                    for h in range(HPC):
                        nc.vector.tensor_copy(
                            kT[h][:, quarter * 512:(quarter + 1) * 512],
                            ab[h][:])
                for qh in range(2):
                    for d in range(KC):
                        for h in range(HPC):
                            nc.tensor.matmul(
                                ab[h][:],
                                wqc[d][:, h * DH:(h + 1) * DH],
                                xc[d][:, qh * QH:(qh + 1) * QH],
                                start=(d == 0), stop=(d == KC - 1))
                    for h in range(HPC):
                        nc.vector.tensor_scalar_add(
                            qT[h][:, qh * QH:(qh + 1) * QH],
                            ab[h][:], bq_sb[:, h:h + 1])

            # ---- attention + V projection + output projection, pipelined.
            with (
                tc.tile_pool(name="pTp", bufs=16) as pTp,
                tc.tile_pool(name="vsb", bufs=8) as vsbp,
                tc.tile_pool(name="rsb", bufs=8) as rsbp,
                tc.tile_pool(name="osb", bufs=4) as osbp,
                tc.tile_pool(name="msk", bufs=8 if masked else 1) as mskp,
                tc.tile_pool(name="ppS", bufs=1, space="PSUM") as ppS,
                tc.tile_pool(name="ppF", bufs=1, space="PSUM") as ppF,
            ):
                st = [ppS.tile([P, 1024], f32, name=f"s{t}") for t in range(2)]
                ft = [ppF.tile([P, 129], f32, name=f"f{t}") for t in range(2)]

                unit_pts = {}   # unit -> list of 8 pT tiles
                sg_counter = [0]

                def emit_scores_group(u, g):
                    """Two scores matmuls (chunks 2g, 2g+1) + one exp."""
                    h, qh = u % HPC, u // HPC
                    s = st[sg_counter[0] % 2]
                    sg_counter[0] += 1
                    for jj in range(2):
                        j = g * 2 + jj
                        nc.tensor.matmul(
                            s[:, jj * 512:(jj + 1) * 512],
                            kT[h][:, j * P:(j + 1) * P],
                            qT[h][:, qh * QH:(qh + 1) * QH],
                            start=True, stop=True)
                        if masked:
                            mt = mskp.tile([P, 512], bf16, name="m")
                            nc.sync.dma_start(
                                mt[:], maskT[j * P:(j + 1) * P,
                                             qh * QH:(qh + 1) * QH])
                            nc.vector.tensor_add(
                                s[:, jj * 512:(jj + 1) * 512],
                                s[:, jj * 512:(jj + 1) * 512], mt[:])
                    pt = pTp.tile([P, 1024], bf16, name="pt")
                    nc.scalar.activation(pt[:], s[:], Exp)
                    unit_pts.setdefault(u, []).append(pt)

                def emit_fused_sub(u, qs, half):
                    """8 fused attn@V+denominator matmuls (one j-half of the
                    16-chunk accumulation); normalize + transpose at the
                    end."""
                    h, qh = u % HPC, u // HPC
                    f = ft[qs % 2]
                    pts = unit_pts[u]
                    for j in range(half * 8, half * 8 + 8):
                        g, jj = j // 2, j % 2
                        nc.tensor.matmul(
                            f[:],
                            pts[g][:, jj * 512 + qs * P:jj * 512 + (qs + 1) * P],
                            vch[j][:, h, 0:129],
                            start=(j == 0), stop=(j == LKC - 1))
                    if half == 1:
                        rc = rsbp.tile([P, 1], f32, name="rc")
                        nc.vector.reciprocal(rc[:], f[:, 128:129])
                        vs = vsbp.tile([P, P], bf16, name="vs")
                        nc.vector.tensor_scalar_mul(vs[:], f[:, 0:128], rc[:])
                        nc.sync.dma_start(
                            valsT[h][:, qh * QH + qs * P:qh * QH + (qs + 1) * P],
                            vs[:], transpose=True)

                def emit_vproj_half(j, half, vtile):
                    """4 of the 8 accumulating V-projection matmuls for key
                    chunk j; copy out on the second half."""
                    lh, jloc = j // 8, j % 8
                    for d in range(half * 4, half * 4 + 4):
                        nc.tensor.matmul(
                            vtile[:],
                            e[lh][d][:, jloc * P:(jloc + 1) * P],
                            wvc[d][:],
                            start=(d == 0), stop=(d == KC - 1))
                    if half == 1:
                        nc.vector.tensor_copy(vch[j][:, :, 0:128], vtile[:])

                def emit_outproj_group(qh, lqc, oh, otile, obuf):
                    """Output projection for one (q-chunk, out-half)."""
                    for h in range(HPC):
                        nc.tensor.matmul(
                            otile[:],
                            valsT[h][:, qh * QH + lqc * P:
                                      qh * QH + (lqc + 1) * P],
                            woch[h][:, oh * 512:(oh + 1) * 512],
                            start=(h == 0), stop=(h == HPC - 1))
                    nc.vector.tensor_copy(obuf[:, oh * 512:(oh + 1) * 512],
                                          otile[:])
                    if oh == 1:
                        lq = qh * 4 + lqc
                        nc.sync.dma_start(
                            out_d[lq * P:(lq + 1) * P, :], obuf[:])

                fillers = deque()

                def pop_fillers(budget, emit_out):
                    while budget > 0 and fillers:
                        item = fillers.popleft()
                        if item[0] == "v":
                            _, j, half, vtile = item
                            emit_vproj_half(j, half, vtile)
                            budget -= 853
                        elif item[0] == "f":
                            _, u, qs, half = item
                            emit_fused_sub(u, qs, half)
                            budget -= 430
                        else:
                            _, qh, lqc, oh = item
                            emit_outproj_group(qh, lqc, oh, *emit_out(qh, lqc, oh))
                            budget -= 853

                # ---- phase 2: units 0-3 (qh0); V projection front-loaded.
                with tc.tile_pool(name="ppV", bufs=1, space="PSUM") as ppV:
                    vt = [ppV.tile([P, HPC, 128], f32, name=f"v{t}")
                          for t in range(2)]
                    for j in range(LKC):
                        for half in range(2):
                            fillers.append(("v", j, half, vt[j % 2]))

                    for u in range(4):
                        for g in range(8):
                            emit_scores_group(u, g)
                            pop_fillers(1800 if u < 2 else 900, None)
                        for qs in range(4):
                            for half in range(2):
                                fillers.append(("f", u, qs, half))
                    # all V work must be emitted inside ppV's scope
                    assert not any(i[0] == "v" for i in fillers)

                # ---- phase 3: units 4-7 (qh1) + output projection.
                with tc.tile_pool(name="ppO", bufs=1, space="PSUM") as ppO:
                    ot = [ppO.tile([P, 512], f32, name=f"o{t}")
                          for t in range(2)]
                    og_counter = [0]
                    obufs = {}

                    def out_args(qh, lqc, oh):
                        key = (qh, lqc)
                        if oh == 0:
                            obufs[key] = osbp.tile([P, D], bf16, name="ob")
                        otile = ot[og_counter[0] % 2]
                        og_counter[0] += 1
                        return otile, obufs[key]

                    for u in range(4, 8):
                        for g in range(8):
                            emit_scores_group(u, g)
                            pop_fillers(900, out_args)
                        for qs in range(4):
                            for half in range(2):
                                fillers.append(("f", u, qs, half))
                        if u == 4:
                            # qh0 fused (F_3) is queued ahead in the deque;
                            # qh0 outproj goes behind it.
                            for lqc in range(4):
                                for oh in range(2):
                                    fillers.append(("o", 0, lqc, oh))
                    # drain: F_7 (and any leftovers), then qh1 outproj.
                    while fillers:
                        pop_fillers(10**9, out_args)
                    for lqc in range(4):
                        for oh in range(2):
                            emit_outproj_group(1, lqc, oh,
                                               *out_args(1, lqc, oh))

    nc.compile()
    return nc


def _get_built(masked):
    if masked not in _BUILT:
        _BUILT[masked] = _build(masked)
    return _BUILT[masked]


def _shard_inputs(inputs, masked):
    import ml_dtypes
    bf16 = ml_dtypes.bfloat16

    x = np.asarray(inputs["mhca_input"], np.float32)
    enc = np.asarray(inputs["encoder_output"], np.float32)
    mask = np.asarray(inputs["cross_mask"], np.float32)
    W_kv = np.asarray(inputs["W_kv"], np.float32)
    W_q = np.asarray(inputs["W_q"], np.float32)
    b_q = np.asarray(inputs["b_q"], np.float32)
    W_o = np.asarray(inputs["W_o"], np.float32)

    scale = 1.0 / math.sqrt(DH)
    in_maps = []
    for c in range(N_CORES):
        b = c // 2
        g = c % 2
        heads = list(range(g * HPC, (g + 1) * HPC))
        sl = slice(g * OQ, (g + 1) * OQ)
        k_rows = np.concatenate(
            [W_kv[h * 2 * DH:h * 2 * DH + DH] for h in heads], 0)
        v_rows = np.concatenate(
            [W_kv[h * 2 * DH + DH:(h + 1) * 2 * DH] for h in heads], 0)
        m = {
            "encT": np.ascontiguousarray(enc[b].T).astype(bf16),
            "xT": np.ascontiguousarray(x[b].T).astype(bf16),
            "wkT": np.ascontiguousarray(k_rows.T).astype(bf16),
            "wvT": np.ascontiguousarray(v_rows.T).astype(bf16),
            "wqT": np.ascontiguousarray((W_q[sl] * scale).T).astype(bf16),
            "woT": np.ascontiguousarray(W_o[:, sl].T).astype(bf16),
            "bq": np.ascontiguousarray((b_q[sl] * scale).reshape(HPC, DH).T),
        }
        if masked:
            m["maskT"] = np.ascontiguousarray(mask[b].T).astype(bf16)
        in_maps.append(m)
    return in_maps


def kernel(mhca_input, encoder_output, cross_mask, W_kv, b_kv, W_q, b_q, W_o,
           b_o):
    from concourse.bass_utils import run_bass_kernel_spmd

    inputs = {
        "mhca_input": mhca_input, "encoder_output": encoder_output,
        "cross_mask": cross_mask, "W_kv": W_kv, "b_kv": b_kv, "W_q": W_q,
        "b_q": b_q, "W_o": W_o,
    }
    b_kv = np.asarray(b_kv, np.float32)
    b_o = np.asarray(b_o, np.float32)
    W_o_np = np.asarray(W_o, np.float32)
    masked = bool(np.any(np.asarray(cross_mask)))
    nc = _get_built(masked)
    in_maps = _shard_inputs(inputs, masked)

    res = run_bass_kernel_spmd(nc, in_maps, core_ids=list(range(N_CORES)))
    outs = [np.asarray(res.results[c]["out"], np.float32)
            for c in range(N_CORES)]
    full = np.stack([outs[2 * b] + outs[2 * b + 1] for b in range(B)], 0)
    # v-bias folds into a constant output bias: attn@(v+bv) = attn@v + bv.
    b_v = np.concatenate([b_kv[h * 2 * DH + DH:(h + 1) * 2 * DH]
                          for h in range(H)], 0)
    bias = b_o + W_o_np @ b_v
    return (full + bias[None, None, :]).astype(np.float32)


# revision 19
# speedup vs baseline: 1.3824x; 1.0711x over previous
"""Multi-head cross-attention on 8 Trainium2 NeuronCores.

Problem shapes (hardcoded): B=4, Ld=1024, Le=2048, d_model=1024, 8 heads x 128.
Sharding: core c handles batch b=c//2 and head-group g=c%2 (4 heads each).
Each core computes q/k/v projections for its heads, attention, and a partial
output projection over its heads' value dims; the host sums the two partial
outputs per batch and adds the bias.

Everything runs in bf16 (inputs converted host-side), matmuls at full PE rate.
Exact algebraic reductions vs the reference:
  - the k bias is dropped: adding q.bk to every score of a query cancels in
    softmax,
  - the v bias folds into a constant output bias (attention weights sum to 1,
    so attn@(v+bv) = attn@v + bv), applied host-side together with b_o,
  - the softmax denominator is computed by the same matmuls as attn@V: the
    moving operand is [v_chunk | ones-column] (129 wide) with exp'd scores as
    the stationary operand, so column 128 of the accumulator is sum(exp) and
    no separate denominator pass is needed.
The fused attn@V produces vals in [q, vd] layout; a DMA-XBAR transpose turns
it into [vd, q] for the output projection, keeping the PE free.

Work is emitted software-pipelined in units of (head, query-half).  Engine
queues are strict FIFO, so emission order is chosen so that no instruction
ever waits on one emitted later: V-projection chunks are front-loaded as
filler behind units 0-1's scores, unit u's fused matmuls are emitted early in
unit u+1 (before u+1's scores can throttle on their exp WAR edges), and the
output projection fills units 5-7.
"""

import math
import sys
from collections import deque

import numpy as np

for _p in ("/opt/trn_rl_repo", "/root/.axon_site/_ro/trn_rl_repo"):
    if _p not in sys.path:
        sys.path.append(_p)

B = 4
LQ = 1024
LK = 2048
D = 1024
H = 8
DH = 128
P = 128
HPC = 4          # heads per core
OQ = HPC * DH    # 512 projected dims per core
KC = D // P      # 8 contraction chunks for projections
LKC = LK // P    # 16 key chunks
QH = 512         # query half
N_CORES = 8

_BUILT = {}


def _build(masked):
    import concourse.bass as bass  # noqa: F401
    import concourse.tile as tile
    import concourse.mybir as mybir
    from concourse import bacc

    f32 = mybir.dt.float32
    bf16 = mybir.dt.bfloat16
    Exp = mybir.ActivationFunctionType.Exp

    nc = bacc.Bacc("TRN2", target_bir_lowering=False, debug=False,
                   num_devices=N_CORES)

    encT = nc.dram_tensor("encT", [D, LK], bf16, kind="ExternalInput").ap()
    xT = nc.dram_tensor("xT", [D, LQ], bf16, kind="ExternalInput").ap()
    wkT = nc.dram_tensor("wkT", [D, OQ], bf16, kind="ExternalInput").ap()
    wvT = nc.dram_tensor("wvT", [D, OQ], bf16, kind="ExternalInput").ap()
    wqT = nc.dram_tensor("wqT", [D, OQ], bf16, kind="ExternalInput").ap()
    woT = nc.dram_tensor("woT", [OQ, D], bf16, kind="ExternalInput").ap()
    bq_d = nc.dram_tensor("bq", [P, HPC], f32, kind="ExternalInput").ap()
    if masked:
        maskT = nc.dram_tensor("maskT", [LK, LQ], bf16,
                               kind="ExternalInput").ap()
    out_d = nc.dram_tensor("out", [LQ, D], bf16, kind="ExternalOutput").ap()

    with tile.TileContext(nc) as tc:
        with tc.tile_pool(name="persist", bufs=1) as persist:
            e = [[persist.tile([P, 1024], bf16, name=f"e{lh}_{d}")
                  for d in range(KC)] for lh in range(2)]
            xc = [persist.tile([P, LQ], bf16, name=f"x{d}") for d in range(KC)]
            wkc = [persist.tile([P, OQ], bf16, name=f"wk{d}") for d in range(KC)]
            wvc = [persist.tile([P, OQ], bf16, name=f"wv{d}") for d in range(KC)]
            wqc = [persist.tile([P, OQ], bf16, name=f"wq{d}") for d in range(KC)]
            woch = [persist.tile([P, D], bf16, name=f"wo{h}")
                    for h in range(HPC)]
            kT = [persist.tile([P, LK], bf16, name=f"kT{h}") for h in range(HPC)]
            qT = [persist.tile([P, LQ], bf16, name=f"qT{h}") for h in range(HPC)]
            # per key-chunk: 4 head blocks of [128 v-dims | ones | pad]
            vch = [persist.tile([P, HPC, 130], bf16, name=f"v{j}")
                   for j in range(LKC)]
            valsT = [persist.tile([P, LQ], bf16, name=f"valsT{h}")
                     for h in range(HPC)]
            bq_sb = persist.tile([P, HPC], f32, name="bq")

            # ---- input DMAs, in consumption order.  The first enc chunk is
            # split so the very first matmul starts sooner; enc chunks
            # alternate between the SP HWDGE and the gpsimd SWDGE paths so
            # descriptor generation pipelines in parallel and the transfer
            # device (360 GB/s) is the only pacer for the cold start.
            nc.sync.dma_start(wkc[0][:], wkT[0:P, :])
            nc.gpsimd.dma_start(e[0][0][:, :512], encT[0:P, :512])
            nc.gpsimd.dma_start(e[0][0][:, 512:], encT[0:P, 512:1024])
            for d in range(1, KC):
                nc.sync.dma_start(wkc[d][:], wkT[d * P:(d + 1) * P, :])
                eng = nc.gpsimd if d % 2 else nc.sync
                eng.dma_start(e[0][d][:], encT[d * P:(d + 1) * P, :1024])
            for d in range(KC):
                nc.sync.dma_start(e[1][d][:], encT[d * P:(d + 1) * P, 1024:])
            nc.sync.dma_start(bq_sb[:], bq_d[:])
            for d in range(KC):
                nc.sync.dma_start(wqc[d][:], wqT[d * P:(d + 1) * P, :])
                nc.gpsimd.dma_start(xc[d][:], xT[d * P:(d + 1) * P, :])
            for d in range(KC):
                nc.sync.dma_start(wvc[d][:], wvT[d * P:(d + 1) * P, :])
            for h in range(HPC):
                nc.sync.dma_start(woch[h][:], woT[h * P:(h + 1) * P, :])

            # ones columns for the fused attn@V / denominator matmuls
            for j in range(LKC):
                nc.vector.memset(vch[j][:], 1.0)

            # ---- K projection then Q projection (4 PSUM banks).  Each K
            # half runs as two (2 quarters x 2 heads) passes so the cold
            # start consumes each arriving enc chunk with 4 matmuls while
            # the remaining work stays available for when DMA lags.
            with tc.tile_pool(name="ppA", bufs=1, space="PSUM") as ppA:
                ab = [ppA.tile([P, 512], f32, name=f"a{t}") for t in range(4)]
                for lh in range(2):
                    for hp in range(2):          # head pair
                        for d in range(KC):
                            for t in range(4):
                                lkq, hh = t // 2, hp * 2 + t % 2
                                nc.tensor.matmul(
                                    ab[t][:],
                                    wkc[d][:, hh * DH:(hh + 1) * DH],
                                    e[lh][d][:, lkq * 512:(lkq + 1) * 512],
                                    start=(d == 0), stop=(d == KC - 1))
                        for t in range(4):
                            lkq, hh = t // 2, hp * 2 + t % 2
                            quarter = lh * 2 + lkq
                            nc.vector.tensor_copy(
                                kT[hh][:, quarter * 512:(quarter + 1) * 512],
                                ab[t][:])
                for qh in range(2):
                    for d in range(KC):
                        for h in range(HPC):
                            nc.tensor.matmul(
                                ab[h][:],
                                wqc[d][:, h * DH:(h + 1) * DH],
                                xc[d][:, qh * QH:(qh + 1) * QH],
                                start=(d == 0), stop=(d == KC - 1))
                    for h in range(HPC):
                        nc.vector.tensor_scalar_add(
                            qT[h][:, qh * QH:(qh + 1) * QH],
                            ab[h][:], bq_sb[:, h:h + 1])

            # ---- attention + V projection + output projection, pipelined.
            with (
                tc.tile_pool(name="pTp", bufs=24) as pTp,
                tc.tile_pool(name="vsb", bufs=8) as vsbp,
                tc.tile_pool(name="rsb", bufs=8) as rsbp,
                tc.tile_pool(name="osb", bufs=4) as osbp,
                tc.tile_pool(name="msk", bufs=8 if masked else 1) as mskp,
                tc.tile_pool(name="ppS", bufs=1, space="PSUM") as ppS,
                tc.tile_pool(name="ppF", bufs=1, space="PSUM") as ppF,
            ):
                st = [ppS.tile([P, 1024], f32, name=f"s{t}") for t in range(2)]
                ft = [ppF.tile([P, 129], f32, name=f"f{t}") for t in range(2)]

                unit_pts = {}   # unit -> list of 8 pT tiles
                sg_counter = [0]

                def emit_scores_group(u, g):
                    """Two scores matmuls (chunks 2g, 2g+1) + one exp."""
                    h, qh = u % HPC, u // HPC
                    s = st[sg_counter[0] % 2]
                    sg_counter[0] += 1
                    for jj in range(2):
                        j = g * 2 + jj
                        nc.tensor.matmul(
                            s[:, jj * 512:(jj + 1) * 512],
                            kT[h][:, j * P:(j + 1) * P],
                            qT[h][:, qh * QH:(qh + 1) * QH],
                            start=True, stop=True)
                        if masked:
                            mt = mskp.tile([P, 512], bf16, name="m")
                            nc.sync.dma_start(
                                mt[:], maskT[j * P:(j + 1) * P,
                                             qh * QH:(qh + 1) * QH])
                            nc.vector.tensor_add(
                                s[:, jj * 512:(jj + 1) * 512],
                                s[:, jj * 512:(jj + 1) * 512], mt[:])
                    pt = pTp.tile([P, 1024], bf16, name="pt")
                    nc.scalar.activation(pt[:], s[:], Exp)
                    unit_pts.setdefault(u, []).append(pt)

                def emit_fused_quarter(u, qs, quarter):
                    """4 fused attn@V+denominator matmuls (one j-quarter of
                    the 16-chunk accumulation); normalize + transpose after
                    the last one."""
                    h, qh = u % HPC, u // HPC
                    f = ft[qs % 2]
                    pts = unit_pts[u]
                    for j in range(quarter * 4, quarter * 4 + 4):
                        g, jj = j // 2, j % 2
                        nc.tensor.matmul(
                            f[:],
                            pts[g][:, jj * 512 + qs * P:jj * 512 + (qs + 1) * P],
                            vch[j][:, h, 0:129],
                            start=(j == 0), stop=(j == LKC - 1))
                    if quarter == 3:
                        rc = rsbp.tile([P, 1], f32, name="rc")
                        nc.vector.reciprocal(rc[:], f[:, 128:129])
                        vs = vsbp.tile([P, P], bf16, name="vs")
                        nc.vector.tensor_scalar_mul(vs[:], f[:, 0:128], rc[:])
                        nc.sync.dma_start(
                            valsT[h][:, qh * QH + qs * P:qh * QH + (qs + 1) * P],
                            vs[:], transpose=True)

                def emit_vproj_half(j, half, vtile):
                    """4 of the 8 accumulating V-projection matmuls for key
                    chunk j; copy out on the second half."""
                    lh, jloc = j // 8, j % 8
                    for d in range(half * 4, half * 4 + 4):
                        nc.tensor.matmul(
                            vtile[:],
                            e[lh][d][:, jloc * P:(jloc + 1) * P],
                            wvc[d][:],
                            start=(d == 0), stop=(d == KC - 1))
                    if half == 1:
                        nc.vector.tensor_copy(vch[j][:, :, 0:128], vtile[:])

                def emit_outproj_group(qh, lqc, oh, otile, obuf):
                    """Output projection for one (q-chunk, out-half)."""
                    for h in range(HPC):
                        nc.tensor.matmul(
                            otile[:],
                            valsT[h][:, qh * QH + lqc * P:
                                      qh * QH + (lqc + 1) * P],
                            woch[h][:, oh * 512:(oh + 1) * 512],
                            start=(h == 0), stop=(h == HPC - 1))
                    nc.vector.tensor_copy(obuf[:, oh * 512:(oh + 1) * 512],
                                          otile[:])
                    lq = qh * 4 + lqc
                    if qh == 1:
                        # per-half DMAs at the tail so the last transfer is
                        # small and starts as soon as its copy lands
                        nc.sync.dma_start(
                            out_d[lq * P:(lq + 1) * P,
                                  oh * 512:(oh + 1) * 512],
                            obuf[:, oh * 512:(oh + 1) * 512])
                    elif oh == 1:
                        nc.sync.dma_start(
                            out_d[lq * P:(lq + 1) * P, :], obuf[:])

                og_counter = [0]
                obufs = {}

                def out_args(qh, lqc, oh):
                    key = (qh, lqc)
                    if oh == 0:
                        obufs[key] = osbp.tile([P, D], bf16, name="ob")
                    otile = out_tiles[og_counter[0] % 2]
                    og_counter[0] += 1
                    return otile, obufs[key]

                def emit_item(item):
                    if item[0] == "v":
                        emit_vproj_half(item[1], item[2], item[3])
                        return 853
                    if item[0] == "f":
                        emit_fused_quarter(item[1], item[2], item[3])
                        return 215
                    _, qh, lqc, oh = item
                    emit_outproj_group(qh, lqc, oh,
                                       *out_args(qh, lqc, oh))
                    return 853

                def run_unit(u, items):
                    """Emit unit u's 8 scores groups with `items` spread
                    across the 8 slots proportionally by estimated time."""
                    total = sum(853 if i[0] != "f" else 215 for i in items)
                    items = deque(items)
                    done = 0
                    for g in range(8):
                        emit_scores_group(u, g)
                        target = total * (g + 1) // 8
                        while items and done < target:
                            done += emit_item(items.popleft())

                def fq(u, qss, quarters):
                    return [("f", u, qs, q) for qs in qss for q in quarters]

                # ---- phase 2: units 0-3 (qh0); V projection as filler
                # (front-loaded: fused quarters need vch complete), then
                # F_0/F_1 spread behind units 2-3.
                with tc.tile_pool(name="ppV", bufs=1, space="PSUM") as ppV:
                    vt = [ppV.tile([P, HPC, 128], f32, name=f"v{t}")
                          for t in range(2)]
                    vitems = [("v", j, half, vt[j % 2])
                              for j in range(LKC) for half in range(2)]
                    run_unit(0, vitems[:14])
                    run_unit(1, vitems[14:28])
                    run_unit(2, vitems[28:] + fq(0, range(4), range(4)))
                    run_unit(3, fq(1, range(4), range(4)) +
                             fq(2, (0, 1), range(4)))

                # ---- phase 3: units 4-7 (qh1) + output projection.
                with tc.tile_pool(name="ppO", bufs=1, space="PSUM") as ppO:
                    out_tiles = [ppO.tile([P, 512], f32, name=f"o{t}")
                                 for t in range(2)]
                    run_unit(4, fq(2, (2, 3), range(4)) +
                             fq(3, range(4), range(4)))
                    run_unit(5, [("o", 0, 0, 0), ("o", 0, 0, 1)] +
                             fq(4, range(4), range(4)))
                    run_unit(6, [("o", 0, 1, 0), ("o", 0, 1, 1)] +
                             fq(5, range(4), range(4)))
                    run_unit(7, [("o", 0, 2, 0), ("o", 0, 2, 1)] +
                             fq(6, range(4), range(4)))
                    # drain: F_7, a reserved qh0 group to cover qh1 transpose
                    # latency, then qh1 outproj.
                    for item in fq(7, range(4), range(4)):
                        emit_item(item)
                    for oh in range(2):
                        emit_item(("o", 0, 3, oh))
                    for lqc in range(4):
                        for oh in range(2):
                            emit_item(("o", 1, lqc, oh))

    nc.compile()
    return nc


def _get_built(masked):
    if masked not in _BUILT:
        _BUILT[masked] = _build(masked)
    return _BUILT[masked]


def _shard_inputs(inputs, masked):
    import ml_dtypes
    bf16 = ml_dtypes.bfloat16

    x = np.asarray(inputs["mhca_input"], np.float32)
    enc = np.asarray(inputs["encoder_output"], np.float32)
    mask = np.asarray(inputs["cross_mask"], np.float32)
    W_kv = np.asarray(inputs["W_kv"], np.float32)
    W_q = np.asarray(inputs["W_q"], np.float32)
    b_q = np.asarray(inputs["b_q"], np.float32)
    W_o = np.asarray(inputs["W_o"], np.float32)

    scale = 1.0 / math.sqrt(DH)
    in_maps = []
    for c in range(N_CORES):
        b = c // 2
        g = c % 2
        heads = list(range(g * HPC, (g + 1) * HPC))
        sl = slice(g * OQ, (g + 1) * OQ)
        k_rows = np.concatenate(
            [W_kv[h * 2 * DH:h * 2 * DH + DH] for h in heads], 0)
        v_rows = np.concatenate(
            [W_kv[h * 2 * DH + DH:(h + 1) * 2 * DH] for h in heads], 0)
        m = {
            "encT": np.ascontiguousarray(enc[b].T).astype(bf16),
            "xT": np.ascontiguousarray(x[b].T).astype(bf16),
            "wkT": np.ascontiguousarray(k_rows.T).astype(bf16),
            "wvT": np.ascontiguousarray(v_rows.T).astype(bf16),
            "wqT": np.ascontiguousarray((W_q[sl] * scale).T).astype(bf16),
            "woT": np.ascontiguousarray(W_o[:, sl].T).astype(bf16),
            "bq": np.ascontiguousarray((b_q[sl] * scale).reshape(HPC, DH).T),
        }
        if masked:
            m["maskT"] = np.ascontiguousarray(mask[b].T).astype(bf16)
        in_maps.append(m)
    return in_maps


def kernel(mhca_input, encoder_output, cross_mask, W_kv, b_kv, W_q, b_q, W_o,
           b_o):
    from concourse.bass_utils import run_bass_kernel_spmd

    inputs = {
        "mhca_input": mhca_input, "encoder_output": encoder_output,
        "cross_mask": cross_mask, "W_kv": W_kv, "b_kv": b_kv, "W_q": W_q,
        "b_q": b_q, "W_o": W_o,
    }
    b_kv = np.asarray(b_kv, np.float32)
    b_o = np.asarray(b_o, np.float32)
    W_o_np = np.asarray(W_o, np.float32)
    masked = bool(np.any(np.asarray(cross_mask)))
    nc = _get_built(masked)
    in_maps = _shard_inputs(inputs, masked)

    res = run_bass_kernel_spmd(nc, in_maps, core_ids=list(range(N_CORES)))
    outs = [np.asarray(res.results[c]["out"], np.float32)
            for c in range(N_CORES)]
    full = np.stack([outs[2 * b] + outs[2 * b + 1] for b in range(B)], 0)
    # v-bias folds into a constant output bias: attn@(v+bv) = attn@v + bv.
    b_v = np.concatenate([b_kv[h * 2 * DH + DH:(h + 1) * 2 * DH]
                          for h in range(H)], 0)
    bias = b_o + W_o_np @ b_v
    return (full + bias[None, None, :]).astype(np.float32)


# revision 40
# speedup vs baseline: 1.3914x; 1.0065x over previous
"""Multi-head cross-attention on 8 Trainium2 NeuronCores.

Problem shapes (hardcoded): B=4, Ld=1024, Le=2048, d_model=1024, 8 heads x 128.
Sharding: core c handles batch b=c//2 and head-group g=c%2 (4 heads each).
Each core computes q/k/v projections for its heads, attention, and a partial
output projection over its heads' value dims; the host sums the two partial
outputs per batch and adds the bias.

Everything runs in bf16 (inputs converted host-side), matmuls at full PE rate.
Exact algebraic reductions vs the reference:
  - the k bias is dropped: adding q.bk to every score of a query cancels in
    softmax,
  - the v bias folds into a constant output bias (attention weights sum to 1,
    so attn@(v+bv) = attn@v + bv), applied host-side together with b_o,
  - the softmax denominator is computed by the same matmuls as attn@V: the
    moving operand is [v_chunk | ones-column] (129 wide) with exp'd scores as
    the stationary operand, so column 128 of the accumulator is sum(exp) and
    no separate denominator pass is needed.
The fused attn@V produces vals in [q, vd] layout; a DMA-XBAR transpose turns
it into [vd, q] for the output projection, keeping the PE free.

Work is emitted software-pipelined in units of (head, query-half).  Engine
queues are strict FIFO, so emission order is chosen so that no instruction
ever waits on one emitted later: V-projection chunks are front-loaded as
filler behind units 0-1's scores, unit u's fused matmuls are emitted early in
unit u+1 (before u+1's scores can throttle on their exp WAR edges), and the
output projection fills units 5-7.
"""

import math
import sys
from collections import deque

import numpy as np

for _p in ("/opt/trn_rl_repo", "/root/.axon_site/_ro/trn_rl_repo"):
    if _p not in sys.path:
        sys.path.append(_p)

B = 4
LQ = 1024
LK = 2048
D = 1024
H = 8
DH = 128
P = 128
HPC = 4          # heads per core
OQ = HPC * DH    # 512 projected dims per core
KC = D // P      # 8 contraction chunks for projections
LKC = LK // P    # 16 key chunks
QH = 512         # query half
N_CORES = 8

_BUILT = {}


def _build(masked):
    import concourse.bass as bass  # noqa: F401
    import concourse.tile as tile
    import concourse.mybir as mybir
    from concourse import bacc

    f32 = mybir.dt.float32
    bf16 = mybir.dt.bfloat16
    Exp = mybir.ActivationFunctionType.Exp

    nc = bacc.Bacc("TRN2", target_bir_lowering=False, debug=False,
                   num_devices=N_CORES)

    encT = nc.dram_tensor("encT", [D, LK], bf16, kind="ExternalInput").ap()
    xT = nc.dram_tensor("xT", [D, LQ], bf16, kind="ExternalInput").ap()
    wkT = nc.dram_tensor("wkT", [D, OQ], bf16, kind="ExternalInput").ap()
    wvT = nc.dram_tensor("wvT", [D, OQ], bf16, kind="ExternalInput").ap()
    wqT = nc.dram_tensor("wqT", [D, OQ], bf16, kind="ExternalInput").ap()
    woT = nc.dram_tensor("woT", [OQ, D], bf16, kind="ExternalInput").ap()
    bq_d = nc.dram_tensor("bq", [P, HPC], f32, kind="ExternalInput").ap()
    if masked:
        maskT = nc.dram_tensor("maskT", [LK, LQ], bf16,
                               kind="ExternalInput").ap()
    out_d = nc.dram_tensor("out", [LQ, D], bf16, kind="ExternalOutput").ap()

    with tile.TileContext(nc) as tc:
        with tc.tile_pool(name="persist", bufs=1) as persist:
            e = [[persist.tile([P, 1024], bf16, name=f"e{lh}_{d}")
                  for d in range(KC)] for lh in range(2)]
            xc = [persist.tile([P, LQ], bf16, name=f"x{d}") for d in range(KC)]
            wkc = [persist.tile([P, OQ], bf16, name=f"wk{d}") for d in range(KC)]
            wvc = [persist.tile([P, OQ], bf16, name=f"wv{d}") for d in range(KC)]
            wqc = [persist.tile([P, OQ], bf16, name=f"wq{d}") for d in range(KC)]
            woch = [persist.tile([P, D], bf16, name=f"wo{h}")
                    for h in range(HPC)]
            kT = [persist.tile([P, LK], bf16, name=f"kT{h}") for h in range(HPC)]
            qT = [persist.tile([P, LQ], bf16, name=f"qT{h}") for h in range(HPC)]
            # per key-chunk: 4 head blocks of [128 v-dims | ones | pad]
            vch = [persist.tile([P, HPC, 130], bf16, name=f"v{j}")
                   for j in range(LKC)]
            valsT = [persist.tile([P, LQ], bf16, name=f"valsT{h}")
                     for h in range(HPC)]
            bq_sb = persist.tile([P, HPC], f32, name="bq")

            # ---- input DMAs, in consumption order.  The first enc chunk is
            # split so the very first matmul starts sooner; enc chunks
            # alternate between the SP HWDGE and the gpsimd SWDGE paths so
            # descriptor generation pipelines in parallel and the transfer
            # device (360 GB/s) is the only pacer for the cold start.
            nc.sync.dma_start(wkc[0][:], wkT[0:P, :])
            nc.gpsimd.dma_start(e[0][0][:, :512], encT[0:P, :512])
            nc.gpsimd.dma_start(e[0][0][:, 512:], encT[0:P, 512:1024])
            for d in range(1, KC):
                nc.sync.dma_start(wkc[d][:], wkT[d * P:(d + 1) * P, :])
                eng = nc.gpsimd if d % 2 else nc.sync
                eng.dma_start(e[0][d][:], encT[d * P:(d + 1) * P, :1024])
            for d in range(KC):
                nc.sync.dma_start(e[1][d][:], encT[d * P:(d + 1) * P, 1024:])
            nc.sync.dma_start(bq_sb[:], bq_d[:])
            for d in range(KC):
                nc.sync.dma_start(wqc[d][:], wqT[d * P:(d + 1) * P, :])
                nc.gpsimd.dma_start(xc[d][:], xT[d * P:(d + 1) * P, :])
            for d in range(KC):
                nc.sync.dma_start(wvc[d][:], wvT[d * P:(d + 1) * P, :])
            for h in range(HPC):
                nc.sync.dma_start(woch[h][:], woT[h * P:(h + 1) * P, :])

            # ones columns for the fused attn@V / denominator matmuls
            for j in range(LKC):
                nc.vector.memset(vch[j][:], 1.0)

            # ---- K projection then Q projection.  Phase 1 owns all 8 PSUM
            # banks, so each K half runs as ONE 8-group pass (2 quarters x 4
            # heads): every arriving enc chunk feeds 8 matmuls, keeping the
            # PE ahead of the 360 GB/s transfer stream from the first chunk.
            with tc.tile_pool(name="ppA", bufs=1, space="PSUM") as ppA:
                ab = [ppA.tile([P, 512], f32, name=f"a{t}") for t in range(8)]
                for lh in range(2):
                    for d in range(KC):
                        for t in range(8):
                            lkq, hh = t // 4, t % 4
                            nc.tensor.matmul(
                                ab[t][:],
                                wkc[d][:, hh * DH:(hh + 1) * DH],
                                e[lh][d][:, lkq * 512:(lkq + 1) * 512],
                                start=(d == 0), stop=(d == KC - 1))
                            if d == KC - 1:
                                # drain each accumulator right behind its
                                # closing matmul so the DVE copies overlap
                                # the remaining matmuls
                                quarter = lh * 2 + lkq
                                nc.vector.tensor_copy(
                                    kT[hh][:, quarter * 512:
                                            (quarter + 1) * 512],
                                    ab[t][:])
                for qh in range(2):
                    for d in range(KC):
                        for h in range(HPC):
                            nc.tensor.matmul(
                                ab[(1 - qh) * 4 + h][:],
                                wqc[d][:, h * DH:(h + 1) * DH],
                                xc[d][:, qh * QH:(qh + 1) * QH],
                                start=(d == 0), stop=(d == KC - 1))
                            if d == KC - 1:
                                # split the bias adds across DVE and the
                                # idle ACT engine so the pool drain that
                                # gates the first scores matmul is short
                                eng = (nc.vector.tensor_scalar_add
                                       if h < 2 else nc.scalar.add)
                                eng(qT[h][:, qh * QH:(qh + 1) * QH],
                                    ab[(1 - qh) * 4 + h][:],
                                    bq_sb[:, h:h + 1])

            # ---- attention + V projection + output projection, pipelined.
            with (
                tc.tile_pool(name="pTp", bufs=24) as pTp,
                tc.tile_pool(name="vsb", bufs=8) as vsbp,
                tc.tile_pool(name="rsb", bufs=8) as rsbp,
                tc.tile_pool(name="osb", bufs=4) as osbp,
                tc.tile_pool(name="msk", bufs=8 if masked else 1) as mskp,
                tc.tile_pool(name="ppS", bufs=1, space="PSUM") as ppS,
                tc.tile_pool(name="ppF", bufs=1, space="PSUM") as ppF,
            ):
                st = [ppS.tile([P, 1024], f32, name=f"s{t}") for t in range(2)]
                ft = [ppF.tile([P, 129], f32, name=f"f{t}") for t in range(2)]

                unit_pts = {}   # unit -> list of 8 pT tiles
                sg_counter = [0]

                def emit_scores_group(u, g):
                    """Two scores matmuls (chunks 2g, 2g+1) + one exp."""
                    h, qh = u % HPC, u // HPC
                    s = st[sg_counter[0] % 2]
                    sg_counter[0] += 1
                    for jj in range(2):
                        j = g * 2 + jj
                        nc.tensor.matmul(
                            s[:, jj * 512:(jj + 1) * 512],
                            kT[h][:, j * P:(j + 1) * P],
                            qT[h][:, qh * QH:(qh + 1) * QH],
                            start=True, stop=True)
                        if masked:
                            mt = mskp.tile([P, 512], bf16, name="m")
                            nc.sync.dma_start(
                                mt[:], maskT[j * P:(j + 1) * P,
                                             qh * QH:(qh + 1) * QH])
                            nc.vector.tensor_add(
                                s[:, jj * 512:(jj + 1) * 512],
                                s[:, jj * 512:(jj + 1) * 512], mt[:])
                    pt = pTp.tile([P, 1024], bf16, name="pt")
                    nc.scalar.activation(pt[:], s[:], Exp)
                    unit_pts.setdefault(u, []).append(pt)

                def emit_fused_quarter(u, qs, quarter):
                    """4 fused attn@V+denominator matmuls (one j-quarter of
                    the 16-chunk accumulation); normalize + transpose after
                    the last one."""
                    h, qh = u % HPC, u // HPC
                    f = ft[qs % 2]
                    pts = unit_pts[u]
                    for j in range(quarter * 4, quarter * 4 + 4):
                        g, jj = j // 2, j % 2
                        nc.tensor.matmul(
                            f[:],
                            pts[g][:, jj * 512 + qs * P:jj * 512 + (qs + 1) * P],
                            vch[j][:, h, 0:129],
                            start=(j == 0), stop=(j == LKC - 1))
                    if quarter == 3:
                        rc = rsbp.tile([P, 1], f32, name="rc")
                        nc.vector.reciprocal(rc[:], f[:, 128:129])
                        vs = vsbp.tile([P, P], bf16, name="vs")
                        nc.vector.tensor_scalar_mul(vs[:], f[:, 0:128], rc[:])
                        nc.sync.dma_start(
                            valsT[h][:, qh * QH + qs * P:qh * QH + (qs + 1) * P],
                            vs[:], transpose=True)

                def emit_vproj_half(j, half, vtile):
                    """4 of the 8 accumulating V-projection matmuls for key
                    chunk j; copy out on the second half."""
                    lh, jloc = j // 8, j % 8
                    for d in range(half * 4, half * 4 + 4):
                        nc.tensor.matmul(
                            vtile[:],
                            e[lh][d][:, jloc * P:(jloc + 1) * P],
                            wvc[d][:],
                            start=(d == 0), stop=(d == KC - 1))
                    if half == 1:
                        nc.vector.tensor_copy(vch[j][:, :, 0:128], vtile[:])

                def emit_outproj_group(qh, lqc, oh, otile, obuf):
                    """Output projection for one (q-chunk, out-half)."""
                    for h in range(HPC):
                        nc.tensor.matmul(
                            otile[:],
                            valsT[h][:, qh * QH + lqc * P:
                                      qh * QH + (lqc + 1) * P],
                            woch[h][:, oh * 512:(oh + 1) * 512],
                            start=(h == 0), stop=(h == HPC - 1))
                    nc.vector.tensor_copy(obuf[:, oh * 512:(oh + 1) * 512],
                                          otile[:])
                    lq = qh * 4 + lqc
                    if qh == 1:
                        # per-half DMAs at the tail so the last transfer is
                        # small and starts as soon as its copy lands
                        nc.sync.dma_start(
                            out_d[lq * P:(lq + 1) * P,
                                  oh * 512:(oh + 1) * 512],
                            obuf[:, oh * 512:(oh + 1) * 512])
                    elif oh == 1:
                        nc.sync.dma_start(
                            out_d[lq * P:(lq + 1) * P, :], obuf[:])

                og_counter = [0]
                obufs = {}

                def out_args(qh, lqc, oh):
                    key = (qh, lqc)
                    if oh == 0:
                        obufs[key] = osbp.tile([P, D], bf16, name="ob")
                    otile = out_tiles[og_counter[0] % 2]
                    og_counter[0] += 1
                    return otile, obufs[key]

                def emit_item(item):
                    if item[0] == "v":
                        emit_vproj_half(item[1], item[2], item[3])
                        return 853
                    if item[0] == "f":
                        emit_fused_quarter(item[1], item[2], item[3])
                        return 215
                    _, qh, lqc, oh = item
                    emit_outproj_group(qh, lqc, oh,
                                       *out_args(qh, lqc, oh))
                    return 853

                def run_unit(u, items):
                    """Emit unit u's 8 scores groups with `items` spread
                    across the 8 slots proportionally by estimated time."""
                    total = sum(853 if i[0] != "f" else 215 for i in items)
                    items = deque(items)
                    done = 0
                    for g in range(8):
                        emit_scores_group(u, g)
                        target = total * (g + 1) // 8
                        while items and done < target:
                            done += emit_item(items.popleft())

                def fq(u, qss, quarters):
                    return [("f", u, qs, q) for qs in qss for q in quarters]

                # ---- phase 2: units 0-3 (qh0); V projection as filler
                # (front-loaded: fused quarters need vch complete), then
                # F_0/F_1 spread behind units 2-3.
                with tc.tile_pool(name="ppV", bufs=1, space="PSUM") as ppV:
                    vt = [ppV.tile([P, HPC, 128], f32, name=f"v{t}")
                          for t in range(2)]
                    vitems = [("v", j, half, vt[j % 2])
                              for j in range(LKC) for half in range(2)]
                    run_unit(0, vitems[:14])
                    run_unit(1, vitems[14:28])
                    run_unit(2, vitems[28:] + fq(0, range(4), range(4)))
                    run_unit(3, fq(1, range(4), range(4)) +
                             fq(2, (0, 1), range(4)))

                # ---- phase 3: units 4-7 (qh1) + output projection.
                with tc.tile_pool(name="ppO", bufs=1, space="PSUM") as ppO:
                    out_tiles = [ppO.tile([P, 512], f32, name=f"o{t}")
                                 for t in range(2)]
                    run_unit(4, fq(2, (2, 3), range(4)) +
                             fq(3, range(4), range(4)))
                    run_unit(5, [("o", 0, 0, 0), ("o", 0, 0, 1)] +
                             fq(4, range(4), range(4)))
                    run_unit(6, [("o", 0, 1, 0), ("o", 0, 1, 1)] +
                             fq(5, range(4), range(4)))
                    run_unit(7, [("o", 0, 2, 0), ("o", 0, 2, 1)] +
                             fq(6, range(4), range(4)))
                    # drain: F_7, a reserved qh0 group to cover qh1 transpose
                    # latency, then qh1 outproj.
                    for item in fq(7, range(4), range(4)):
                        emit_item(item)
                    for oh in range(2):
                        emit_item(("o", 0, 3, oh))
                    for lqc in range(4):
                        for oh in range(2):
                            emit_item(("o", 1, lqc, oh))

    nc.compile()
    return nc


def _get_built(masked):
    if masked not in _BUILT:
        _BUILT[masked] = _build(masked)
    return _BUILT[masked]


def _shard_inputs(inputs, masked):
    import ml_dtypes
    bf16 = ml_dtypes.bfloat16

    x = np.asarray(inputs["mhca_input"], np.float32)
    enc = np.asarray(inputs["encoder_output"], np.float32)
    mask = np.asarray(inputs["cross_mask"], np.float32)
    W_kv = np.asarray(inputs["W_kv"], np.float32)
    W_q = np.asarray(inputs["W_q"], np.float32)
    b_q = np.asarray(inputs["b_q"], np.float32)
    W_o = np.asarray(inputs["W_o"], np.float32)

    scale = 1.0 / math.sqrt(DH)
    in_maps = []
    for c in range(N_CORES):
        b = c // 2
        g = c % 2
        heads = list(range(g * HPC, (g + 1) * HPC))
        sl = slice(g * OQ, (g + 1) * OQ)
        k_rows = np.concatenate(
            [W_kv[h * 2 * DH:h * 2 * DH + DH] for h in heads], 0)
        v_rows = np.concatenate(
            [W_kv[h * 2 * DH + DH:(h + 1) * 2 * DH] for h in heads], 0)
        m = {
            "encT": np.ascontiguousarray(enc[b].T).astype(bf16),
            "xT": np.ascontiguousarray(x[b].T).astype(bf16),
            "wkT": np.ascontiguousarray(k_rows.T).astype(bf16),
            "wvT": np.ascontiguousarray(v_rows.T).astype(bf16),
            "wqT": np.ascontiguousarray((W_q[sl] * scale).T).astype(bf16),
            "woT": np.ascontiguousarray(W_o[:, sl].T).astype(bf16),
            "bq": np.ascontiguousarray((b_q[sl] * scale).reshape(HPC, DH).T),
        }
        if masked:
            m["maskT"] = np.ascontiguousarray(mask[b].T).astype(bf16)
        in_maps.append(m)
    return in_maps


def kernel(mhca_input, encoder_output, cross_mask, W_kv, b_kv, W_q, b_q, W_o,
           b_o):
    from concourse.bass_utils import run_bass_kernel_spmd

    inputs = {
        "mhca_input": mhca_input, "encoder_output": encoder_output,
        "cross_mask": cross_mask, "W_kv": W_kv, "b_kv": b_kv, "W_q": W_q,
        "b_q": b_q, "W_o": W_o,
    }
    b_kv = np.asarray(b_kv, np.float32)
    b_o = np.asarray(b_o, np.float32)
    W_o_np = np.asarray(W_o, np.float32)
    masked = bool(np.any(np.asarray(cross_mask)))
    nc = _get_built(masked)
    in_maps = _shard_inputs(inputs, masked)

    res = run_bass_kernel_spmd(nc, in_maps, core_ids=list(range(N_CORES)))
    outs = [np.asarray(res.results[c]["out"], np.float32)
            for c in range(N_CORES)]
    full = np.stack([outs[2 * b] + outs[2 * b + 1] for b in range(B)], 0)
    # v-bias folds into a constant output bias: attn@(v+bv) = attn@v + bv.
    b_v = np.concatenate([b_kv[h * 2 * DH + DH:(h + 1) * 2 * DH]
                          for h in range(H)], 0)
    bias = b_o + W_o_np @ b_v
    return (full + bias[None, None, :]).astype(np.float32)
